# revision 33
# baseline (speedup 1.0000x reference)
"""Multi-head attention Trainium2 kernel (8-core SPMD).

Sharding: core c -> batch b = c//4, head-group g = c%4 (4 heads each).
Each core computes partial_out[S, D] = attn(4 heads) @ Wo[rows of its heads].
Host sums the 4 partials per batch (unshard of Wo's contracted input dim) + bo.

Layout strategy (per core, S=2048 D=1024 DK=64, 4 local heads = 2 pairs):
  - host passes x^T [D, S] bf16 so every projection contracts d on partitions.
  - Q/K proj: pair-stacked lhsT=W[d,128] -> QT/KT [2*64, S] psum, copied to
    bf16 "duplicated" per-head tiles (rows 0:64 and 64:128 both hold head h)
    so scores can row-tile two K=64 matmuls concurrently in the PE array.
  - V proj: V4[t, 4*65] bf16, per head 64 cols of V plus a ones column ->
    PV matmul yields [65, s]: rows 0:64 unnormalized out^T, row 64 = rowsum.
  - scores^T[t, s] psum [128, 1024] tiles -> ScalarE exp (scale=1/8 folded)
    -> U bf16; PV accumulates t-outer chasing the exps.
  - normalize via DVE reciprocal + GpSimd partition_broadcast + DVE multiply.
  - Wo: pair-stacked OT2 [128, s] tiles, K=128 matmuls, bf16 partial out
    (host sums the four partials per batch in fp32 and adds bo).
  - emission order is tuned for the in-order engines: pair-0 projections
    first, pair-1 projections re-stream x from DRAM and overlap heads 1/0,
    per-head softmax normalization is deferred past the next head's first
    t-iteration, and the Wo epilogue pipelines through both PSUM pools.
"""

import os
import sys

import numpy as np

sys.path.insert(0, "/opt/trn_rl_repo")

import ml_dtypes

BF16 = ml_dtypes.bfloat16

_CACHE = {}


def _build_nc(S, D, DK, NH, with_bias=True):
    import concourse.bass as bass
    import concourse.mybir as mybir
    import concourse.tile as tile
    from concourse import bacc

    bf = mybir.dt.bfloat16
    f32 = mybir.dt.float32
    P = 128
    NPAIR = NH // 2
    KT = D // P            # contraction tiles for projections
    TT = S // P            # t-chunks
    SC = S // 512          # 512-wide s-chunks
    SH = S // 1024         # 1024-wide s-halves per t-chunk

    nc = bacc.Bacc("TRN2", target_bir_lowering=False, debug=False)

    xqT = nc.declare_dram_parameter("xqT", [D, S], bf, isOutput=False)
    xkT = nc.declare_dram_parameter("xkT", [D, S], bf, isOutput=False)
    xvT = nc.declare_dram_parameter("xvT", [D, S], bf, isOutput=False)
    wq2 = nc.declare_dram_parameter("wq2", [NPAIR, P, D], bf, isOutput=False)
    wk2 = nc.declare_dram_parameter("wk2", [NPAIR, P, D], bf, isOutput=False)
    wv4 = nc.declare_dram_parameter("wv4", [P, KT * NH * DK], bf, isOutput=False)
    bq2 = nc.declare_dram_parameter("bq2", [NPAIR, 1, P], bf, isOutput=False)
    bk2 = nc.declare_dram_parameter("bk2", [NPAIR, 1, P], bf, isOutput=False)
    bv4 = nc.declare_dram_parameter("bv4", [1, NH * DK], bf, isOutput=False)
    wo2 = nc.declare_dram_parameter("wo2", [NPAIR, P, D], bf, isOutput=False)
    out_d = nc.declare_dram_parameter("out", [S, D], bf, isOutput=True)

    EXP = mybir.ActivationFunctionType.Exp
    scale = 1.0 / np.sqrt(DK)

    with tile.TileContext(nc) as tc:
        with (
            tc.tile_pool(name="consts", bufs=1) as consts,
            tc.tile_pool(name="wp", bufs=1) as wp,
            tc.tile_pool(name="xt", bufs=12) as xt,
            tc.tile_pool(name="qk", bufs=1) as qkp,
            tc.tile_pool(name="vb", bufs=1) as vbp,
            tc.tile_pool(name="up", bufs=16) as up,
            tc.tile_pool(name="ot", bufs=1) as otp,
            tc.tile_pool(name="sm", bufs=4) as smp,
            tc.tile_pool(name="outp", bufs=3) as outp,
            tc.tile_pool(name="psb", bufs=2, space="PSUM") as psb,
            tc.tile_pool(name="pss", bufs=4, space="PSUM") as pss,
        ):
            # constants
            ones_s = consts.tile([1, S], bf, tag="ones_s")
            nc.vector.memset(ones_s[:], 1.0)

            # weights to SBUF (wq first; the rest after the x loads
            # so the first projection's inputs hit DMA earliest)
            wq_sb, wk_sb, bq_sb, bk_sb, wo_sb = [], [], [], [], []
            for p in range(NPAIR):
                wqt = wp.tile([P, D], bf, tag=f"wq{p}")
                nc.sync.dma_start(out=wqt[:], in_=wq2[p])
                wq_sb.append(wqt)
                bqt = wp.tile([1, P], bf, tag=f"bq{p}")
                if with_bias:
                    nc.sync.dma_start(out=bqt[:], in_=bq2[p])
                bq_sb.append(bqt)

            # persistent per-head dup-stacked QT/KT tiles
            QTd = [qkp.tile([P, S], bf, tag=f"qtd{h}", name=f"qtd{h}") for h in range(NH)]
            KTd = [qkp.tile([P, S], bf, tag=f"ktd{h}", name=f"ktd{h}") for h in range(NH)]

            # x tiles: loaded once, resident during their projection
            def load_x(x_dram, nm):
                ts = []
                for k in range(KT):
                    t = xt.tile([P, S], bf, tag="x", name=f"x{nm}{k}")
                    nc.sync.dma_start(out=t[:], in_=x_dram[k * P : (k + 1) * P, :])
                    ts.append(t)
                return ts

            xq_sb = load_x(xqT, "q")
            for p in range(NPAIR):
                wkt = wp.tile([P, D], bf, tag=f"wk{p}")
                nc.sync.dma_start(out=wkt[:], in_=wk2[p])
                wk_sb.append(wkt)
                bkt = wp.tile([1, P], bf, tag=f"bk{p}")
                if with_bias:
                    nc.sync.dma_start(out=bkt[:], in_=bk2[p])
                bk_sb.append(bkt)
            xk_sb = load_x(xkT, "k")

            # Q/K projection for one pair, k-outer over resident x tiles
            def proj_pair(x_sb, w_sb_t, b_sb_t, dst, p, nm):
                h0, h1 = 2 * p, 2 * p + 1
                acc = [
                    pss.tile([P, 512], f32, tag="acc", bufs=4, name=f"{nm}ps{p}_{s}")
                    for s in range(SC)
                ]
                for k in range(KT):
                    for s in range(SC):
                        nc.tensor.matmul(
                            acc[s][:],
                            w_sb_t[:, k * P : (k + 1) * P],
                            x_sb[k][:, s * 512 : (s + 1) * 512],
                            start=(k == 0),
                            stop=(not with_bias and k == KT - 1),
                        )
                for s in range(SC):
                    if with_bias:
                        nc.tensor.matmul(
                            acc[s][:],
                            b_sb_t[0:1, :],
                            ones_s[0:1, s * 512 : (s + 1) * 512],
                            start=False,
                            stop=True,
                        )
                    sl = slice(s * 512, (s + 1) * 512)
                    nc.vector.tensor_copy(dst[h0][0:64, sl], acc[s][0:64, :])
                    nc.scalar.copy(dst[h1][64:128, sl], acc[s][64:128, :])

            def dup(dst, h):
                # self-duplicate across partition halves (SBUF->SBUF DMA)
                if h % 2 == 0:
                    nc.sync.dma_start(out=dst[h][64:128, :], in_=dst[h][0:64, :])
                else:
                    nc.sync.dma_start(out=dst[h][0:64, :], in_=dst[h][64:128, :])

            proj_pair(xq_sb, wq_sb[0], bq_sb[0], QTd, 0, "q")
            proj_pair(xk_sb, wk_sb[0], bk_sb[0], KTd, 0, "k")
            dup(KTd, 1)
            dup(QTd, 1)
            dup(KTd, 0)
            dup(QTd, 0)
            wv_sb = wp.tile([P, KT * NH * DK], bf, tag="wv")
            nc.sync.dma_start(out=wv_sb[:], in_=wv4[:])
            bv_sb = wp.tile([1, NH * DK], bf, tag="bv")
            if with_bias:
                nc.sync.dma_start(out=bv_sb[:], in_=bv4[:])
            xv_sb = load_x(xvT, "v")
            for p in range(NPAIR):
                wot = wp.tile([P, D], bf, tag=f"wo{p}")
                nc.sync.dma_start(out=wot[:], in_=wo2[p])
                wo_sb.append(wot)

            # ---- V projection: V4b[t] = [128, NH*65] (V cols + ones col) ----
            # k-outer in 4-t-chunk passes so pass 0 pipelines with xv DMAs
            V4b = [None] * TT
            NV = NH * DK
            for tg in range(TT // 4):
                vpss = []
                for t in range(4 * tg, 4 * tg + 4):
                    vpss.append(
                        pss.tile([P, NV], f32, tag="acc", bufs=4, name=f"vps{t}")
                    )
                for k in range(KT):
                    for i, t in enumerate(range(4 * tg, 4 * tg + 4)):
                        nc.tensor.matmul(
                            vpss[i][:],
                            xv_sb[k][:, t * P : (t + 1) * P],
                            wv_sb[:, k * NV : (k + 1) * NV],
                            start=(k == 0),
                            stop=(not with_bias and k == KT - 1),
                        )
                for i, t in enumerate(range(4 * tg, 4 * tg + 4)):
                    if with_bias:
                        nc.tensor.matmul(
                            vpss[i][:],
                            ones_s[0:1, t * P : (t + 1) * P],
                            bv_sb[0:1, :],
                            start=False,
                            stop=True,
                        )
                    vt = vbp.tile(
                        [P, NH * (DK + 1)], bf, tag=f"v4b{t}", name=f"v4b{t}"
                    )
                    nc.vector.tensor_copy(
                        vt.rearrange("p (h e) -> p h e", e=DK + 1)[:, :, 0:DK],
                        vpss[i].rearrange("p (h d) -> p h d", d=DK),
                    )
                    nc.vector.memset(
                        vt.rearrange("p (h e) -> p h e", e=DK + 1)[:, :, DK : DK + 1],
                        1.0,
                    )
                    V4b[t] = vt

            # ---- attention per local head ----
            OT2 = [otp.tile([P, S], bf, tag=f"ot{p}", name=f"ot{p}") for p in range(NPAIR)]
            pending_norm = [None]

            def attn_head(h):
                p, r = h // 2, h % 2
                pv = [
                    pss.tile([DK + 1, 512], f32, tag="acc", bufs=4, name=f"pv{h}_{s}")
                    for s in range(SC)
                ]
                for t in range(TT):
                    if t == 1 and pending_norm[0] is not None:
                        pending_norm[0]()
                        pending_norm[0] = None
                    tsl = slice(t * P, (t + 1) * P)
                    us = []
                    for half in range(SH):
                        sc_t = psb.tile(
                            [P, 1024], f32, tag="sc", bufs=2, name=f"sc{h}_{t}_{half}"
                        )
                        for j in range(2):
                            s0 = half * 1024 + j * 512
                            rp = slice(64 * j, 64 * (j + 1))
                            nc.tensor.matmul(
                                sc_t[:, j * 512 : (j + 1) * 512],
                                KTd[h][rp, tsl],
                                QTd[h][rp, s0 : s0 + 512],
                                start=True,
                                stop=True,
                                tile_position=(64 * j, 0),
                            )
                        ut = up.tile([P, 1024], bf, tag="u", name=f"u{h}_{t}_{half}")
                        nc.scalar.activation(ut[:], sc_t[:], EXP, scale=float(scale))
                        us.append(ut)
                    for s in range(SC):
                        nc.tensor.matmul(
                            pv[s][:],
                            V4b[t][:, h * (DK + 1) : (h + 1) * (DK + 1)],
                            us[s // 2][:, (s % 2) * 512 : (s % 2 + 1) * 512],
                            start=(t == 0),
                            stop=(t == TT - 1),
                        )
                # normalize: rows 0:64 / row 64 (emitted after the next
                # head's first t-iteration so its scores outrank this chain)
                def make_norm(h=h, p=p, r=r, pv=pv):
                    def norm():
                        if r == 1:
                            ottmp = smp.tile(
                                [64, S], bf, tag="ottmp", bufs=2, name=f"otmp{h}"
                            )
                        for s in range(SC):
                            sl = slice(s * 512, (s + 1) * 512)
                            rsb = smp.tile(
                                [1, 512], f32, tag="r", bufs=4, name=f"r{h}_{s}"
                            )
                            nc.vector.reciprocal(rsb[:], pv[s][64:65, :])
                            bcs = smp.tile(
                                [64, 512], f32, tag="bcs", bufs=4, name=f"bcs{h}_{s}"
                            )
                            nc.gpsimd.partition_broadcast(
                                bcs[:], rsb[0:1, :], channels=64
                            )
                            osb = smp.tile(
                                [64, 512], bf, tag="o", bufs=8, name=f"o{h}_{s}"
                            )
                            nc.vector.tensor_copy(osb[:], pv[s][0:64, :])
                            dst = OT2[p][0:64, sl] if r == 0 else ottmp[:, sl]
                            nc.vector.tensor_mul(dst, osb[:], bcs[:])
                        if r == 1:
                            nc.sync.dma_start(out=OT2[p][64:128, :], in_=ottmp[:, :])
                    return norm

                if pending_norm[0] is not None:
                    pending_norm[0]()
                pending_norm[0] = make_norm()

            attn_head(1)
            xq2_sb = load_x(xqT, "q2")
            proj_pair(xq2_sb, wq_sb[1], bq_sb[1], QTd, 1, "q")
            xk2_sb = load_x(xkT, "k2")
            proj_pair(xk2_sb, wk_sb[1], bk_sb[1], KTd, 1, "k")
            dup(KTd, 3)
            dup(QTd, 3)
            dup(KTd, 2)
            dup(QTd, 2)
            attn_head(0)
            attn_head(3)
            attn_head(2)
            pending_norm[0]()

            # ---- output projection (K = NPAIR*128 accumulated in psum) ----
            for m in range(S // P):
                msl = slice(m * P, (m + 1) * P)
                if m % 2 == 0:
                    # big-pool unit: one [128, 1024] psum tile for both halves
                    wopb = psb.tile([P, D], f32, tag="sc", bufs=2, name=f"wopb{m}")
                    for dj in range(D // 512):
                        for p in range(NPAIR):
                            nc.tensor.matmul(
                                wopb[:, dj * 512 : (dj + 1) * 512],
                                OT2[p][:, msl],
                                wo_sb[p][:, dj * 512 : (dj + 1) * 512],
                                start=(p == 0),
                                stop=(p == NPAIR - 1),
                            )
                    ot_b = outp.tile([P, D], bf, tag="outt", name=f"outt{m}")
                    nc.vector.tensor_copy(ot_b[:], wopb[:])
                    nc.sync.dma_start(out=out_d[msl, :], in_=ot_b[:])
                else:
                    for dj in range(D // 512):
                        wops = pss.tile(
                            [P, 512], f32, tag="acc", bufs=4, name=f"wops{m}_{dj}"
                        )
                        for p in range(NPAIR):
                            nc.tensor.matmul(
                                wops[:],
                                OT2[p][:, msl],
                                wo_sb[p][:, dj * 512 : (dj + 1) * 512],
                                start=(p == 0),
                                stop=(p == NPAIR - 1),
                            )
                        ot_t = outp.tile(
                            [P, 512], bf, tag="outt2", name=f"outt{m}_{dj}"
                        )
                        nc.scalar.copy(ot_t[:], wops[:])
                        nc.sync.dma_start(
                            out=out_d[msl, dj * 512 : (dj + 1) * 512], in_=ot_t[:]
                        )

    nc.finalize()
    return nc


def _prep_core_inputs(query, key, value, Wq, bq, Wk, bk, Wv, bv, Wo, b, g, NH, DK):
    """Host-side shard prep for core (b, g): transpose+cast, pack weights."""
    D = query.shape[2]
    h0 = g * NH
    sl = slice(h0, h0 + NH)
    Wq_g, Wk_g, Wv_g = Wq[sl], Wk[sl], Wv[sl]
    bq_g, bk_g, bv_g = bq[sl], bk[sl], bv[sl]
    NPAIR = NH // 2
    P = 128
    KT = D // P

    def pack_pair(W, bias):
        # [NPAIR, 128, D]: pair p cols = heads (2p, 2p+1) concat; k-major free
        w = np.concatenate(
            [
                np.concatenate([W[2 * p], W[2 * p + 1]], axis=1)[None]
                for p in range(NPAIR)
            ],
            axis=0,
        )  # [NPAIR, D, 128]
        w = w.reshape(NPAIR, KT, P, P).transpose(0, 2, 1, 3).reshape(NPAIR, P, D)
        bb = np.concatenate(
            [
                np.concatenate([bias[2 * p], bias[2 * p + 1]])[None, None]
                for p in range(NPAIR)
            ],
            axis=0,
        )  # [NPAIR, 1, 128]
        return w.astype(BF16), bb.astype(BF16)

    wq2, bq2 = pack_pair(Wq_g, bq_g)
    wk2, bk2 = pack_pair(Wk_g, bk_g)
    wv = np.concatenate([Wv_g[i] for i in range(NH)], axis=1)  # [D, NH*DK]
    NV = NH * DK
    wv4 = wv.reshape(KT, P, NV).transpose(1, 0, 2).reshape(P, KT * NV).astype(BF16)
    bv4 = np.concatenate([bv_g[i] for i in range(NH)])[None].astype(BF16)
    wo2 = (
        Wo[h0 * DK : (h0 + NH) * DK]
        .reshape(NPAIR, P, D)
        .astype(BF16)
    )
    return {
        "xqT": np.ascontiguousarray(query[b].T).astype(BF16),
        "xkT": np.ascontiguousarray(key[b].T).astype(BF16),
        "xvT": np.ascontiguousarray(value[b].T).astype(BF16),
        "wq2": wq2,
        "wk2": wk2,
        "wv4": wv4,
        "bq2": bq2,
        "bk2": bk2,
        "bv4": bv4,
        "wo2": wo2,
    }


def kernel(query, key, value, Wq, bq, Wk, bk, Wv, bv, Wo, bo, _trace=False):
    from concourse.bass_utils import run_bass_kernel_spmd

    query = np.asarray(query, np.float32)
    key = np.asarray(key, np.float32)
    value = np.asarray(value, np.float32)
    B, S, D = query.shape
    H, _, DK = np.asarray(Wq).shape
    NCORE = 8
    GROUPS = NCORE // B
    NH = H // GROUPS

    with_bias = bool(
        np.any(np.asarray(bq)) or np.any(np.asarray(bk)) or np.any(np.asarray(bv))
    )
    ck = ("nc", with_bias)
    if ck not in _CACHE:
        _CACHE[ck] = _build_nc(S, D, DK, NH, with_bias=with_bias)
    nc = _CACHE[ck]

    in_maps = []
    for c in range(NCORE):
        b, g = c // GROUPS, c % GROUPS
        in_maps.append(
            _prep_core_inputs(
                np.asarray(query), np.asarray(key), np.asarray(value),
                np.asarray(Wq), np.asarray(bq), np.asarray(Wk), np.asarray(bk),
                np.asarray(Wv), np.asarray(bv), np.asarray(Wo), b, g, NH, DK,
            )
        )

    res = run_bass_kernel_spmd(nc, in_maps, list(range(NCORE)), trace=_trace)
    out = np.zeros((B, S, D), np.float32)
    for c in range(NCORE):
        out[c // GROUPS] += np.asarray(res.results[c]["out"], np.float32)
    out += np.asarray(bo, np.float32)[None, None, :]
    if _trace:
        _CACHE["last_results"] = res
    return out


# revision 34
# speedup vs baseline: 1.0257x; 1.0257x over previous
"""Multi-head attention Trainium2 kernel (8-core SPMD).

Sharding: core c -> batch b = c//4, head-group g = c%4 (4 heads each).
Each core computes partial_out[S, D] = attn(4 heads) @ Wo[rows of its heads].
Host sums the 4 partials per batch (unshard of Wo's contracted input dim) + bo.

Layout strategy (per core, S=2048 D=1024 DK=64, 4 local heads = 2 pairs):
  - host passes x^T [D, S] bf16 so every projection contracts d on partitions.
  - Q/K proj: pair-stacked lhsT=W[d,128] -> QT/KT [2*64, S] psum, copied to
    bf16 "duplicated" per-head tiles (rows 0:64 and 64:128 both hold head h)
    so scores can row-tile two K=64 matmuls concurrently in the PE array.
  - V proj: V4[t, 4*65] bf16, per head 64 cols of V plus a ones column ->
    PV matmul yields [65, s]: rows 0:64 unnormalized out^T, row 64 = rowsum.
  - scores^T[t, s] psum [128, 1024] tiles -> ScalarE exp (scale=1/8 folded)
    -> U bf16; PV accumulates t-outer chasing the exps.
  - normalize via DVE reciprocal + GpSimd partition_broadcast + DVE multiply.
  - Wo: pair-stacked OT2 [128, s] tiles, K=128 matmuls, bf16 partial out
    (host sums the four partials per batch in fp32 and adds bo).
  - emission order is tuned for the in-order engines: pair-0 projections
    first, pair-1 projections re-stream x from DRAM and overlap heads 1/0,
    per-head softmax normalization is deferred past the next head's first
    t-iteration, and the Wo epilogue pipelines through both PSUM pools.
"""

import os
import sys

import numpy as np

sys.path.insert(0, "/opt/trn_rl_repo")

import ml_dtypes

BF16 = ml_dtypes.bfloat16

_CACHE = {}


def _build_nc(S, D, DK, NH, with_bias=True):
    import concourse.bass as bass
    import concourse.mybir as mybir
    import concourse.tile as tile
    from concourse import bacc

    bf = mybir.dt.bfloat16
    f32 = mybir.dt.float32
    P = 128
    NPAIR = NH // 2
    KT = D // P            # contraction tiles for projections
    TT = S // P            # t-chunks
    SC = S // 512          # 512-wide s-chunks
    SH = S // 1024         # 1024-wide s-halves per t-chunk

    nc = bacc.Bacc("TRN2", target_bir_lowering=False, debug=False)

    xqT = nc.declare_dram_parameter("xqT", [D, S], bf, isOutput=False)
    xkT = nc.declare_dram_parameter("xkT", [D, S], bf, isOutput=False)
    xvT = nc.declare_dram_parameter("xvT", [D, S], bf, isOutput=False)
    wq2 = nc.declare_dram_parameter("wq2", [NPAIR, P, D], bf, isOutput=False)
    wk2 = nc.declare_dram_parameter("wk2", [NPAIR, P, D], bf, isOutput=False)
    wv4 = nc.declare_dram_parameter("wv4", [P, KT * NH * DK], bf, isOutput=False)
    bq2 = nc.declare_dram_parameter("bq2", [NPAIR, 1, P], bf, isOutput=False)
    bk2 = nc.declare_dram_parameter("bk2", [NPAIR, 1, P], bf, isOutput=False)
    bv4 = nc.declare_dram_parameter("bv4", [1, NH * DK], bf, isOutput=False)
    wo2 = nc.declare_dram_parameter("wo2", [NPAIR, P, D], bf, isOutput=False)
    out_d = nc.declare_dram_parameter("out", [S, D], bf, isOutput=True)

    EXP = mybir.ActivationFunctionType.Exp
    scale = 1.0 / np.sqrt(DK)

    with tile.TileContext(nc) as tc:
        with (
            tc.tile_pool(name="consts", bufs=1) as consts,
            tc.tile_pool(name="wp", bufs=1) as wp,
            tc.tile_pool(name="xt", bufs=12) as xt,
            tc.tile_pool(name="qk", bufs=1) as qkp,
            tc.tile_pool(name="vb", bufs=1) as vbp,
            tc.tile_pool(name="up", bufs=16) as up,
            tc.tile_pool(name="ot", bufs=1) as otp,
            tc.tile_pool(name="sm", bufs=4) as smp,
            tc.tile_pool(name="outp", bufs=3) as outp,
            tc.tile_pool(name="psb", bufs=2, space="PSUM") as psb,
            tc.tile_pool(name="pss", bufs=4, space="PSUM") as pss,
        ):
            # constants
            ones_s = consts.tile([1, S], bf, tag="ones_s")
            nc.vector.memset(ones_s[:], 1.0)

            # weights to SBUF (wq first; the rest after the x loads
            # so the first projection's inputs hit DMA earliest)
            wq_sb, wk_sb, bq_sb, bk_sb, wo_sb = [], [], [], [], []
            for p in range(NPAIR):
                wqt = wp.tile([P, D], bf, tag=f"wq{p}")
                nc.sync.dma_start(out=wqt[:], in_=wq2[p])
                wq_sb.append(wqt)
                bqt = wp.tile([1, P], bf, tag=f"bq{p}")
                if with_bias:
                    nc.sync.dma_start(out=bqt[:], in_=bq2[p])
                bq_sb.append(bqt)

            # persistent per-head dup-stacked QT/KT tiles
            QTd = [qkp.tile([P, S], bf, tag=f"qtd{h}", name=f"qtd{h}") for h in range(NH)]
            KTd = [qkp.tile([P, S], bf, tag=f"ktd{h}", name=f"ktd{h}") for h in range(NH)]

            # x tiles: loaded once, resident during their projection
            def load_x(x_dram, nm):
                ts = []
                for k in range(KT):
                    t = xt.tile([P, S], bf, tag="x", name=f"x{nm}{k}")
                    nc.sync.dma_start(out=t[:], in_=x_dram[k * P : (k + 1) * P, :])
                    ts.append(t)
                return ts

            xq_sb = load_x(xqT, "q")
            for p in range(NPAIR):
                wkt = wp.tile([P, D], bf, tag=f"wk{p}")
                nc.sync.dma_start(out=wkt[:], in_=wk2[p])
                wk_sb.append(wkt)
                bkt = wp.tile([1, P], bf, tag=f"bk{p}")
                if with_bias:
                    nc.sync.dma_start(out=bkt[:], in_=bk2[p])
                bk_sb.append(bkt)
            xk_sb = load_x(xkT, "k")

            # Both-pair projection in ONE x pass: pair 0 accumulates in the
            # small psum pool (4x [128,512]), pair 1 in the big pool
            # (2x [128,1024]) which is idle until the first scores.
            def proj_both(x_sb, w0, w1, b0, b1, dst, nm):
                acc = [
                    pss.tile([P, 512], f32, tag="acc", bufs=4, name=f"{nm}ps0_{s}")
                    for s in range(SC)
                ]
                acb = [
                    psb.tile([P, 1024], f32, tag="sc", bufs=2, name=f"{nm}ps1_{j}")
                    for j in range(SH)
                ]
                for k in range(KT):
                    ksl = slice(k * P, (k + 1) * P)
                    for s in range(SC):
                        nc.tensor.matmul(
                            acc[s][:],
                            w0[:, ksl],
                            x_sb[k][:, s * 512 : (s + 1) * 512],
                            start=(k == 0),
                            stop=(not with_bias and k == KT - 1),
                        )
                    for s in range(SC):
                        nc.tensor.matmul(
                            acb[s // 2][:, (s % 2) * 512 : (s % 2 + 1) * 512],
                            w1[:, ksl],
                            x_sb[k][:, s * 512 : (s + 1) * 512],
                            start=(k == 0),
                            stop=(not with_bias and k == KT - 1),
                        )
                for s in range(SC):
                    sl = slice(s * 512, (s + 1) * 512)
                    jsl = slice((s % 2) * 512, (s % 2 + 1) * 512)
                    if with_bias:
                        nc.tensor.matmul(
                            acc[s][:],
                            b0[0:1, :],
                            ones_s[0:1, sl],
                            start=False,
                            stop=True,
                        )
                        nc.tensor.matmul(
                            acb[s // 2][:, jsl],
                            b1[0:1, :],
                            ones_s[0:1, sl],
                            start=False,
                            stop=True,
                        )
                    nc.vector.tensor_copy(dst[0][0:64, sl], acc[s][0:64, :])
                    nc.scalar.copy(dst[1][64:128, sl], acc[s][64:128, :])
                    nc.vector.tensor_copy(dst[2][0:64, sl], acb[s // 2][0:64, jsl])
                    nc.scalar.copy(dst[3][64:128, sl], acb[s // 2][64:128, jsl])

            def dup(dst, h):
                # self-duplicate across partition halves (SBUF->SBUF DMA)
                if h % 2 == 0:
                    nc.sync.dma_start(out=dst[h][64:128, :], in_=dst[h][0:64, :])
                else:
                    nc.sync.dma_start(out=dst[h][0:64, :], in_=dst[h][64:128, :])

            proj_both(xq_sb, wq_sb[0], wq_sb[1], bq_sb[0], bq_sb[1], QTd, "q")
            proj_both(xk_sb, wk_sb[0], wk_sb[1], bk_sb[0], bk_sb[1], KTd, "k")
            dup(KTd, 1)
            dup(QTd, 1)
            dup(KTd, 0)
            dup(QTd, 0)
            dup(KTd, 3)
            dup(QTd, 3)
            dup(KTd, 2)
            dup(QTd, 2)
            wv_sb = wp.tile([P, KT * NH * DK], bf, tag="wv")
            nc.sync.dma_start(out=wv_sb[:], in_=wv4[:])
            bv_sb = wp.tile([1, NH * DK], bf, tag="bv")
            if with_bias:
                nc.sync.dma_start(out=bv_sb[:], in_=bv4[:])
            xv_sb = load_x(xvT, "v")
            for p in range(NPAIR):
                wot = wp.tile([P, D], bf, tag=f"wo{p}")
                nc.sync.dma_start(out=wot[:], in_=wo2[p])
                wo_sb.append(wot)

            # ---- V projection: V4b[t] = [128, NH*65] (V cols + ones col) ----
            # k-outer in 4-t-chunk passes so pass 0 pipelines with xv DMAs
            V4b = [None] * TT
            NV = NH * DK
            for tg in range(TT // 4):
                vpss = []
                for t in range(4 * tg, 4 * tg + 4):
                    vpss.append(
                        pss.tile([P, NV], f32, tag="acc", bufs=4, name=f"vps{t}")
                    )
                for k in range(KT):
                    for i, t in enumerate(range(4 * tg, 4 * tg + 4)):
                        nc.tensor.matmul(
                            vpss[i][:],
                            xv_sb[k][:, t * P : (t + 1) * P],
                            wv_sb[:, k * NV : (k + 1) * NV],
                            start=(k == 0),
                            stop=(not with_bias and k == KT - 1),
                        )
                for i, t in enumerate(range(4 * tg, 4 * tg + 4)):
                    if with_bias:
                        nc.tensor.matmul(
                            vpss[i][:],
                            ones_s[0:1, t * P : (t + 1) * P],
                            bv_sb[0:1, :],
                            start=False,
                            stop=True,
                        )
                    vt = vbp.tile(
                        [P, NH * (DK + 1)], bf, tag=f"v4b{t}", name=f"v4b{t}"
                    )
                    nc.vector.tensor_copy(
                        vt.rearrange("p (h e) -> p h e", e=DK + 1)[:, :, 0:DK],
                        vpss[i].rearrange("p (h d) -> p h d", d=DK),
                    )
                    nc.vector.memset(
                        vt.rearrange("p (h e) -> p h e", e=DK + 1)[:, :, DK : DK + 1],
                        1.0,
                    )
                    V4b[t] = vt

            # ---- attention per local head ----
            OT2 = [otp.tile([P, S], bf, tag=f"ot{p}", name=f"ot{p}") for p in range(NPAIR)]
            pending_norm = [None]

            def attn_head(h):
                p, r = h // 2, h % 2
                pv = [
                    pss.tile([DK + 1, 512], f32, tag="acc", bufs=4, name=f"pv{h}_{s}")
                    for s in range(SC)
                ]
                for t in range(TT):
                    if t == 1 and pending_norm[0] is not None:
                        pending_norm[0]()
                        pending_norm[0] = None
                    tsl = slice(t * P, (t + 1) * P)
                    us = []
                    for half in range(SH):
                        sc_t = psb.tile(
                            [P, 1024], f32, tag="sc", bufs=2, name=f"sc{h}_{t}_{half}"
                        )
                        for j in range(2):
                            s0 = half * 1024 + j * 512
                            rp = slice(64 * j, 64 * (j + 1))
                            nc.tensor.matmul(
                                sc_t[:, j * 512 : (j + 1) * 512],
                                KTd[h][rp, tsl],
                                QTd[h][rp, s0 : s0 + 512],
                                start=True,
                                stop=True,
                                tile_position=(64 * j, 0),
                            )
                        ut = up.tile([P, 1024], bf, tag="u", name=f"u{h}_{t}_{half}")
                        nc.scalar.activation(ut[:], sc_t[:], EXP, scale=float(scale))
                        us.append(ut)
                    for s in range(SC):
                        nc.tensor.matmul(
                            pv[s][:],
                            V4b[t][:, h * (DK + 1) : (h + 1) * (DK + 1)],
                            us[s // 2][:, (s % 2) * 512 : (s % 2 + 1) * 512],
                            start=(t == 0),
                            stop=(t == TT - 1),
                        )
                # normalize: rows 0:64 / row 64 (emitted after the next
                # head's first t-iteration so its scores outrank this chain)
                def make_norm(h=h, p=p, r=r, pv=pv):
                    def norm():
                        if r == 1:
                            ottmp = smp.tile(
                                [64, S], bf, tag="ottmp", bufs=2, name=f"otmp{h}"
                            )
                        for s in range(SC):
                            sl = slice(s * 512, (s + 1) * 512)
                            rsb = smp.tile(
                                [1, 512], f32, tag="r", bufs=4, name=f"r{h}_{s}"
                            )
                            nc.vector.reciprocal(rsb[:], pv[s][64:65, :])
                            bcs = smp.tile(
                                [64, 512], f32, tag="bcs", bufs=4, name=f"bcs{h}_{s}"
                            )
                            nc.gpsimd.partition_broadcast(
                                bcs[:], rsb[0:1, :], channels=64
                            )
                            osb = smp.tile(
                                [64, 512], bf, tag="o", bufs=8, name=f"o{h}_{s}"
                            )
                            nc.vector.tensor_copy(osb[:], pv[s][0:64, :])
                            dst = OT2[p][0:64, sl] if r == 0 else ottmp[:, sl]
                            nc.vector.tensor_mul(dst, osb[:], bcs[:])
                        if r == 1:
                            nc.sync.dma_start(out=OT2[p][64:128, :], in_=ottmp[:, :])
                    return norm

                if pending_norm[0] is not None:
                    pending_norm[0]()
                pending_norm[0] = make_norm()

            attn_head(1)
            attn_head(0)
            attn_head(3)
            attn_head(2)
            pending_norm[0]()

            # ---- output projection (K = NPAIR*128 accumulated in psum) ----
            for m in range(S // P):
                msl = slice(m * P, (m + 1) * P)
                if m % 2 == 0:
                    # big-pool unit: one [128, 1024] psum tile for both halves
                    wopb = psb.tile([P, D], f32, tag="sc", bufs=2, name=f"wopb{m}")
                    for dj in range(D // 512):
                        for p in range(NPAIR):
                            nc.tensor.matmul(
                                wopb[:, dj * 512 : (dj + 1) * 512],
                                OT2[p][:, msl],
                                wo_sb[p][:, dj * 512 : (dj + 1) * 512],
                                start=(p == 0),
                                stop=(p == NPAIR - 1),
                            )
                    ot_b = outp.tile([P, D], bf, tag="outt", name=f"outt{m}")
                    nc.vector.tensor_copy(ot_b[:], wopb[:])
                    nc.sync.dma_start(out=out_d[msl, :], in_=ot_b[:])
                else:
                    for dj in range(D // 512):
                        wops = pss.tile(
                            [P, 512], f32, tag="acc", bufs=4, name=f"wops{m}_{dj}"
                        )
                        for p in range(NPAIR):
                            nc.tensor.matmul(
                                wops[:],
                                OT2[p][:, msl],
                                wo_sb[p][:, dj * 512 : (dj + 1) * 512],
                                start=(p == 0),
                                stop=(p == NPAIR - 1),
                            )
                        ot_t = outp.tile(
                            [P, 512], bf, tag="outt2", name=f"outt{m}_{dj}"
                        )
                        nc.scalar.copy(ot_t[:], wops[:])
                        nc.sync.dma_start(
                            out=out_d[msl, dj * 512 : (dj + 1) * 512], in_=ot_t[:]
                        )

    nc.finalize()
    return nc


def _prep_core_inputs(query, key, value, Wq, bq, Wk, bk, Wv, bv, Wo, b, g, NH, DK):
    """Host-side shard prep for core (b, g): transpose+cast, pack weights."""
    D = query.shape[2]
    h0 = g * NH
    sl = slice(h0, h0 + NH)
    Wq_g, Wk_g, Wv_g = Wq[sl], Wk[sl], Wv[sl]
    bq_g, bk_g, bv_g = bq[sl], bk[sl], bv[sl]
    NPAIR = NH // 2
    P = 128
    KT = D // P

    def pack_pair(W, bias):
        # [NPAIR, 128, D]: pair p cols = heads (2p, 2p+1) concat; k-major free
        w = np.concatenate(
            [
                np.concatenate([W[2 * p], W[2 * p + 1]], axis=1)[None]
                for p in range(NPAIR)
            ],
            axis=0,
        )  # [NPAIR, D, 128]
        w = w.reshape(NPAIR, KT, P, P).transpose(0, 2, 1, 3).reshape(NPAIR, P, D)
        bb = np.concatenate(
            [
                np.concatenate([bias[2 * p], bias[2 * p + 1]])[None, None]
                for p in range(NPAIR)
            ],
            axis=0,
        )  # [NPAIR, 1, 128]
        return w.astype(BF16), bb.astype(BF16)

    wq2, bq2 = pack_pair(Wq_g, bq_g)
    wk2, bk2 = pack_pair(Wk_g, bk_g)
    wv = np.concatenate([Wv_g[i] for i in range(NH)], axis=1)  # [D, NH*DK]
    NV = NH * DK
    wv4 = wv.reshape(KT, P, NV).transpose(1, 0, 2).reshape(P, KT * NV).astype(BF16)
    bv4 = np.concatenate([bv_g[i] for i in range(NH)])[None].astype(BF16)
    wo2 = (
        Wo[h0 * DK : (h0 + NH) * DK]
        .reshape(NPAIR, P, D)
        .astype(BF16)
    )
    return {
        "xqT": np.ascontiguousarray(query[b].T).astype(BF16),
        "xkT": np.ascontiguousarray(key[b].T).astype(BF16),
        "xvT": np.ascontiguousarray(value[b].T).astype(BF16),
        "wq2": wq2,
        "wk2": wk2,
        "wv4": wv4,
        "bq2": bq2,
        "bk2": bk2,
        "bv4": bv4,
        "wo2": wo2,
    }


def kernel(query, key, value, Wq, bq, Wk, bk, Wv, bv, Wo, bo, _trace=False):
    from concourse.bass_utils import run_bass_kernel_spmd

    query = np.asarray(query, np.float32)
    key = np.asarray(key, np.float32)
    value = np.asarray(value, np.float32)
    B, S, D = query.shape
    H, _, DK = np.asarray(Wq).shape
    NCORE = 8
    GROUPS = NCORE // B
    NH = H // GROUPS

    with_bias = bool(
        np.any(np.asarray(bq)) or np.any(np.asarray(bk)) or np.any(np.asarray(bv))
    )
    ck = ("nc", with_bias)
    if ck not in _CACHE:
        _CACHE[ck] = _build_nc(S, D, DK, NH, with_bias=with_bias)
    nc = _CACHE[ck]

    in_maps = []
    for c in range(NCORE):
        b, g = c // GROUPS, c % GROUPS
        in_maps.append(
            _prep_core_inputs(
                np.asarray(query), np.asarray(key), np.asarray(value),
                np.asarray(Wq), np.asarray(bq), np.asarray(Wk), np.asarray(bk),
                np.asarray(Wv), np.asarray(bv), np.asarray(Wo), b, g, NH, DK,
            )
        )

    res = run_bass_kernel_spmd(nc, in_maps, list(range(NCORE)), trace=_trace)
    out = np.zeros((B, S, D), np.float32)
    for c in range(NCORE):
        out[c // GROUPS] += np.asarray(res.results[c]["out"], np.float32)
    out += np.asarray(bo, np.float32)[None, None, :]
    if _trace:
        _CACHE["last_results"] = res
    return out
